# revision 21
# baseline (speedup 1.0000x reference)
"""Trainium2 Bass kernel for nn_MoEINR: SIREN MoE implicit neural repr.

Pipeline per point: NeRF positional encoding -> SIREN encoder (2 sine layers +
relu bottleneck residual block) -> policy sine net + softmax gate over 7
experts -> 7 SIREN expert MLPs evaluated densely -> probability-weighted sum.

Strategy: pure data parallel over B=65536 points across 8 cores (8192
points/core, 8 super-tiles of 1024).  All activations feature-major
[feat, batch]; every GEMM is lhsT.T @ rhs with weights pre-transposed and
pre-scaled on the host.  Everything runs fp16 on the PE (1 cycle/row, 1024
moving): x is fed as an fp16 hi+lo pair so even the positional-encoding
matmul (which needs ~22 mantissa bits) is exact via two accumulating fp16
matmuls.  SIREN sines need range reduction (ScalarE Sin table is only valid
in [-pi,pi]): weights are pre-scaled by 30/2pi so matmuls produce q =
z*30/(2pi) in period units, then a fused custom DVE op computes
2pi*(q + b - round(q + b)) via the magic-number rounding trick (fp32 PSUM in,
fp16 out), and ACT Sin evaluates it.  The residual join runs on the PE (fp16
identity matmul accumulated with res3) + ACT relu.  Gate logits and expert
preds share one PSUM tile (partition rows 32:39 / 0:7).  Softmax/exp runs as
a second phase after all tiles so the ACT table set switches only once.
"""
import os
import sys

sys.path.insert(0, "/opt/trn_rl_repo")

import numpy as np

import bass_rust
import concourse.bass as bass
import concourse.mybir as mybir
from concourse import tile
from concourse import dve_ops as dops
from concourse.dve_spec import Spec, Src0, Src1, C0, C1, C2, relu as dve_relu_node
from concourse.dve_uop import DveOpSpec
from concourse.dve_spec import lower as dve_lower, _has_src1 as dve_has_src1
from concourse.bass_utils import run_bass_kernel_spmd

F32 = mybir.dt.float32
F16 = mybir.dt.float16    # 1 cycle/row PE matmul, ~1e-3 rel err (13x margin)
ALU = mybir.AluOpType
ACTF = mybir.ActivationFunctionType

NCORES = 8
B = 65536
BC = B // NCORES          # 8192 points per core
SN = 1024                 # super-tile batch (fp16 moving-operand max)
NST = BC // SN            # super-tiles per core (8)
TWO_PI = float(2.0 * np.pi)
SCL = np.float32(30.0 / TWO_PI)   # radians -> periods prescale for sine layers
MAGIC = np.float32(1.5 * 2 ** 23)  # fp32 round-to-nearest-int via add/sub

# ---------------------------------------------------------------------------
# Tile framework workarounds: this walrus build accepts at most ONE sync-wait
# per instruction; Tile attaches one wait per dependent proc.  Split them.
# ---------------------------------------------------------------------------
_wsplit_counter = [0]


def _split_multiwaits(ordered):
    for bb_name, insts in ordered.items():
        i = 0
        while i < len(insts):
            inst = insts[i]
            si = inst.sync_info
            waits = list(si.on_wait) if si is not None and si.on_wait else []
            if len(waits) > 1:
                keep = waits[-1]
                extras = waits[:-1]
                while len(si.on_wait) > 0:
                    si.on_wait.pop()
                si.on_wait.append(keep)
                for w in extras:
                    _wsplit_counter[0] += 1
                    nop = mybir.InstNoOp(name=f"wsplit-{_wsplit_counter[0]}")
                    nop.engine = inst.engine
                    nop.bass_nofuse = True
                    nop.sync_info = mybir.SyncInfo(on_wait=[w], on_update=[])
                    insts.insert(i, nop)
                    i += 1
            i += 1


class _SplittingClockWait:
    def __init__(self, tc, ordered):
        self._inner = bass_rust.TileClockWait(tc, ordered)
        self._ordered = ordered

    def assign_waits(self, start_bb_name):
        r = self._inner.assign_waits(start_bb_name)
        _split_multiwaits(self._ordered)
        return r

    def __getattr__(self, name):
        return getattr(self._inner, name)


tile.TileClockWait = _SplittingClockWait


class TC(tile.TileContext):
    """TileContext whose tail drain emits one wait per instruction."""

    def _drain_and_barrier(self, tick_clock, wait_clock):
        nc = self.nc
        collector = nc.sync.nop(nofuse=True)
        wait_clock.add_sem_waits(
            collector.ins, bass_rust.ScopedClock({None: tick_clock.global_clock})
        )
        si = collector.ins.sync_info
        waits = list(si.on_wait) if si is not None and si.on_wait else []
        if len(waits) > 1:
            id_to_handle = {h.num: h for h in self.sems.allocated().values()}
            extras = waits[1:]
            while len(si.on_wait) > 1:
                si.on_wait.pop()
            for w in extras:
                assert w.wait_mode == "sem-ge-imm", w.wait_mode
                nc.sync.wait_ge(id_to_handle[w.id], w.wait_value)
        nc.sync.drain()
        nc.all_engine_barrier()
        assert self.sems is not None
        popped = nc._tile_sem_poison_stack.pop()
        assert popped is self._sem_poison
        nc.clear_and_free_semaphores(list(self.sems.allocated().values()))
        nc.all_engine_barrier()


# ---------------------------------------------------------------------------
# Custom DVE op (uop tables are generated at compile time from the Spec).
# SIN_RED: out = (q - ((q + C0) - C0)) * C2  with C0 = MAGIC + bias_periods,
#          C2 = 2pi  ->  full sine-argument range reduction in ONE DVE op.
# ---------------------------------------------------------------------------


def _register_op(name, spec):
    if name in dops._SUB_OPCODE_FOR_NAME:
        return next(o for o in dops.OPS if o.name == name)
    opcode = max(dops._SUB_OPCODE_FOR_NAME.values()) + 1
    assert opcode < 0x20
    op = dops.DveOp(name, spec, subdim=False, uops_sha={})
    for ver in ("v3", "v4"):
        try:
            uops = dve_lower(spec, ver=ver)
        except Exception:
            continue
        s = DveOpSpec(name=name, opcode=opcode, uops=uops,
                      rd1_en=dve_has_src1(spec)).sha(ver)
        op.uops_sha[ver] = s
    dops.OPS.append(op)
    dops.CUSTOM_DVE_SPECS[name] = spec
    dops._SUB_OPCODE_FOR_NAME[name] = opcode
    return op


def _ref_sin_red(in0, in1, s0, s1, imm2):
    p = np.float32(in0.astype(np.float32) + np.float32(s0))
    r = np.float32(np.float32(p + np.float32(s1)) - np.float32(s1))
    return ((p - r) * np.float32(imm2)).astype(np.float32)


_p_node = Src0 + C0   # p = q + bias (C0 = bias AP, C1 = MAGIC imm, C2 = 2pi)
SIN_RED = _register_op(
    "ANT_SIN_RANGE_RED",
    Spec(body=(_p_node - ((_p_node + C1) - C1)) * C2, reference=_ref_sin_red),
)


# ---------------------------------------------------------------------------
# Host-side weight preprocessing
# ---------------------------------------------------------------------------


def _prepare(inputs):
    f = lambda a: np.asarray(a, dtype=np.float32)
    h = np.float16
    d = {}
    xt = np.ascontiguousarray(f(inputs["x"]).T)       # [4,B] fp32
    x16 = xt.astype(h)
    d["x16"] = x16                                    # hi fp16
    d["xlo"] = (xt - x16.astype(np.float32)).astype(h)  # lo fp16 (|.|<2^-12)

    # positional encoding: q[i*16+j] = x_i * 2^(j%8) / 2 (periods);
    # cos rows (j>=8) get +0.25 period via the C0 bias.  Powers of two are
    # exact in fp16; x arrives as hi+lo fp16 so q is fp32-exact.
    pe_w = np.zeros((4, 64), h)
    for i in range(4):
        for j in range(8):
            pe_w[i, i * 16 + j] = 2.0 ** j / 2.0
            pe_w[i, i * 16 + 8 + j] = 2.0 ** j / 2.0
    d["pe_w"] = pe_w

    d["enc1_w"] = np.ascontiguousarray((f(inputs["enc_s1_w"]) * SCL).T.astype(h))
    d["enc2_w"] = np.ascontiguousarray((f(inputs["enc_s2_w"]) * SCL).T.astype(h))
    r1t = f(inputs["res_fc1_w"]).T                                        # [256,128]
    d["res1_w"] = np.ascontiguousarray(
        np.concatenate([r1t[0:128], r1t[128:256]], axis=1).astype(h))     # [128,256]
    d["res2_w"] = np.ascontiguousarray(f(inputs["res_fc2_w"]).T.astype(h))
    d["res3_w"] = np.ascontiguousarray(f(inputs["res_fc3_w"]).T.astype(h))
    d["pol1_w"] = np.ascontiguousarray((f(inputs["pol_s1_w"]) * SCL).T.astype(h))
    d["pol2_w"] = np.ascontiguousarray((f(inputs["pol_s2_w"]) * SCL).T.astype(h))
    gt = f(inputs["gate_w"]).T                                            # [384,7]
    d["gate_wp"] = np.ascontiguousarray(
        np.concatenate([gt[0:128], gt[128:256], gt[256:384]], axis=1).astype(h))
    d["ident"] = np.eye(128, dtype=h)                 # residual add via PE

    w1 = np.zeros((128, 7 * 4 * 128), np.float32)
    w2 = np.zeros((128, 7 * 4 * 128), np.float32)
    for e in range(7):
        t1 = (f(inputs["exp_s1_w"][e]) * SCL).T      # [256,256] (in,out)
        t2 = (f(inputs["exp_s2_w"][e]) * SCL).T
        for kc in range(2):
            for mc in range(2):
                off = ((e * 2 + kc) * 2 + mc) * 128
                w1[:, off:off + 128] = t1[kc * 128:(kc + 1) * 128,
                                          mc * 128:(mc + 1) * 128]
                w2[:, off:off + 128] = t2[kc * 128:(kc + 1) * 128,
                                          mc * 128:(mc + 1) * 128]
    d["w1p"] = w1.astype(h)
    d["w2p"] = w2.astype(h)

    finw = f(inputs["exp_fin_w"])                    # [7,1,256]
    finp = np.zeros((128, 14 * 7), np.float32)
    for e in range(7):
        for kc in range(2):
            blk = e * 2 + kc
            finp[:, blk * 7 + e] = finw[e, 0, kc * 128:(kc + 1) * 128]
    d["finp"] = finp.astype(h)

    sumw = np.zeros((128, 4), np.float32)
    for p in range(128):
        if p % 32 < 7:
            sumw[p, p // 32] = 1.0
    d["sumw"] = sumw

    # C0 constants / biases, packed column-wise into one [128, NCOL] tensor
    bias = {
        "h1": f(inputs["enc_s1_b"]) * SCL,
        "h2": f(inputs["enc_s2_b"]) * SCL,
        "pf1": f(inputs["pol_s1_b"]) * SCL,
        "pf2": f(inputs["pol_s2_b"]) * SCL,
        "r1": f(inputs["res_fc1_b"]),
        "r2": f(inputs["res_fc2_b"]),
        "r3": f(inputs["res_fc3_b"]),
    }
    cols = {}
    cv = []

    def addcol(name, vec128):
        cols[name] = len(cv)
        v = np.zeros(128, np.float32)
        v[: len(vec128)] = vec128
        cv.append(v)

    pe_c0 = np.zeros(64, np.float32)
    for i in range(4):
        pe_c0[i * 16 + 8: i * 16 + 16] += 0.25
    addcol("pe", pe_c0)
    addcol("h1", bias["h1"])
    addcol("h2a", bias["h2"][0:128])
    addcol("h2b", bias["h2"][128:256])
    addcol("pf1", bias["pf1"])
    addcol("pf2", bias["pf2"])
    addcol("r1", bias["r1"])
    addcol("r2", bias["r2"])
    addcol("r3a", bias["r3"][0:128])
    addcol("r3b", bias["r3"][128:256])
    gbr = np.zeros(128, np.float32)
    fbr = np.zeros(128, np.float32)
    gb = f(inputs["gate_b"])
    fb = f(inputs["exp_fin_b"]).reshape(-1)
    for j in range(4):
        gbr[32 * j: 32 * j + 7] = gb
        fbr[32 * j: 32 * j + 7] = fb
    addcol("gb", gbr)
    addcol("fb", fbr)
    e1b = f(inputs["exp_s1_b"]) * SCL                # [7,256]
    e2b = f(inputs["exp_s2_b"]) * SCL
    for e in range(7):
        addcol(f"s1_{e}a", e1b[e, 0:128])
        addcol(f"s1_{e}b", e1b[e, 128:256])
        addcol(f"s2_{e}a", e2b[e, 0:128])
        addcol(f"s2_{e}b", e2b[e, 128:256])
    d["cvec"] = np.ascontiguousarray(np.stack(cv, axis=1))   # [128, ncol]

    flags = {
        "fb_any": bool(np.any(fb != 0)),
    }
    return d, cols, flags


# ---------------------------------------------------------------------------
# Bass kernel builder
# ---------------------------------------------------------------------------


def _build(cols, flags):
    nc = bass.Bass()
    P = {}
    shapes = {
        "x16": [4, BC], "xlo": [4, BC], "pe_w": [4, 64], "enc1_w": [64, 128],
        "enc2_w": [128, 256], "res1_w": [128, 256], "res2_w": [128, 128],
        "res3_w": [128, 256], "pol1_w": [4, 128], "pol2_w": [128, 128],
        "gate_wp": [128, 21], "w1p": [128, 3584], "w2p": [128, 3584],
        "finp": [128, 98], "sumw": [128, 4], "cvec": [128, len(cols)],
        "ident": [128, 128],
    }
    F32_NAMES = ("sumw", "cvec")
    dt_of = {n: (F32 if n in F32_NAMES else F16) for n in shapes}
    for n, s in shapes.items():
        P[n] = nc.dram_tensor(n, s, dt_of[n], kind="ExternalInput")
    ydram = nc.dram_tensor("y", [NST, SN], F32, kind="ExternalOutput")

    with TC(nc) as tc:
        with (
            tc.tile_pool(name="wp", bufs=1) as wp,
            tc.tile_pool(name="ap", bufs=1) as ap,
            tc.tile_pool(name="pp", bufs=3, space="PSUM") as pp,
            tc.tile_pool(name="pg", bufs=1, space="PSUM") as pg_pool,
        ):
            W = {}
            for n in shapes:
                W[n] = wp.tile(shapes[n], dt_of[n], tag=n, name=n)
                nc.sync.dma_start(W[n][:], P[n][:])

            def c0(name, rows=128):
                c = cols[name]
                return W["cvec"][0:rows, c:c + 1]

            Lall = [wp.tile([128, SN], F32, tag=f"Lall{q}", name=f"Lall{q}")
                    for q in range(2)]
            Pall = [wp.tile([128, SN], F32, tag=f"Pall{q}", name=f"Pall{q}")
                    for q in range(2)]
            for q in range(2):
                nc.vector.memset(Lall[q][:], 0.0)
                nc.vector.memset(Pall[q][:], 0.0)

            def mm(out, lhsT, rhs, start=True, stop=True):
                # ISA limit: matmul free dim <= 512 (one PSUM bank of fp32).
                # Split the moving dim; each half is its own accum region so
                # start/stop flags carry over per half.
                n = rhs.shape[-1]
                if n <= 512:
                    nc.tensor.matmul(out, lhsT, rhs, start=start, stop=stop)
                    return
                assert n % 512 == 0
                for hh in range(n // 512):
                    sl = slice(hh * 512, (hh + 1) * 512)
                    nc.tensor.matmul(out[:, sl], lhsT, rhs[:, sl],
                                     start=start, stop=stop)

            def sin_red(m_out, q_psum, c0_ap):
                nc.vector._custom_dve(SIN_RED, out=m_out, in0=q_psum,
                                      s0=c0_ap, s1=float(MAGIC), imm2=TWO_PI)

            def emit_trunk(t, ctx, stage_sink=None):
                """Emit trunk for super-tile t.  If stage_sink is a list,
                append 7 stage closures instead of emitting inline."""
                x16s = W["x16"][:, t * SN:(t + 1) * SN]
                xlos = W["xlo"][:, t * SN:(t + 1) * SN]

                def s_pe():
                    ang = pp.tile([64, SN], F32, tag="pp")
                    mm(ang[:], W["pe_w"][:], x16s, stop=False)
                    mm(ang[:], W["pe_w"][:], xlos, start=False)
                    m_pe = ap.tile([64, SN], F16, tag="m5", bufs=4)
                    sin_red(m_pe[:], ang[:], c0("pe", 64))
                    pe_sb = ap.tile([64, SN], F16, tag="a512", bufs=6)
                    nc.scalar.activation(pe_sb[:], m_pe[:], ACTF.Sin)
                    ctx["pe_sb"] = pe_sb

                def s_h1():
                    h1p = pp.tile([128, SN], F32, tag="pp")
                    mm(h1p[:], W["enc1_w"][:], ctx["pe_sb"][:])
                    m_h1 = ap.tile([128, SN], F16, tag="m5", bufs=4)
                    sin_red(m_h1[:], h1p[:], c0("h1"))
                    h1_sb = ap.tile([128, SN], F16, tag="a512", bufs=6)
                    nc.scalar.activation(h1_sb[:], m_h1[:], ACTF.Sin)
                    ctx["h1_sb"] = h1_sb

                def s_h2():
                    m_h2 = ap.tile([128, 2 * SN], F16, tag="m", bufs=6)
                    for mc, cn in ((0, "h2a"), (1, "h2b")):
                        h2p = pp.tile([128, SN], F32, tag="pp")
                        mm(h2p[:], W["enc2_w"][:, mc * 128:(mc + 1) * 128],
                           ctx["h1_sb"][:])
                        sin_red(m_h2[:, mc * SN:(mc + 1) * SN], h2p[:],
                                c0(cn))
                    h2_sb = ap.tile([128, 2 * SN], F16, tag="h2", bufs=2)
                    nc.scalar.activation(h2_sb[:], m_h2[:], ACTF.Sin)
                    ctx["h2_sb"] = h2_sb

                def s_r12():
                    h2_sb = ctx["h2_sb"]
                    r1p = pp.tile([128, SN], F32, tag="pp")
                    mm(r1p[:], W["res1_w"][:, 0:128], h2_sb[:, 0:SN],
                       stop=False)
                    mm(r1p[:], W["res1_w"][:, 128:256], h2_sb[:, SN:2 * SN],
                       start=False)
                    r1_sb = ap.tile([128, SN], F16, tag="a512", bufs=6)
                    nc.scalar.activation(r1_sb[:], r1p[:], ACTF.Relu,
                                         bias=c0("r1"))
                    r2p = pp.tile([128, SN], F32, tag="pp")
                    mm(r2p[:], W["res2_w"][:], r1_sb[:])
                    r2_sb = ap.tile([128, SN], F16, tag="a512", bufs=6)
                    nc.scalar.activation(r2_sb[:], r2p[:], ACTF.Relu,
                                         bias=c0("r2"))
                    ctx["r2_sb"] = r2_sb

                def s_r3():
                    h2_sb, r2_sb = ctx["h2_sb"], ctx["r2_sb"]
                    encf = ap.tile([128, 2 * SN], F16, tag="encf", bufs=2)
                    for mc, cn in ((0, "r3a"), (1, "r3b")):
                        sl = slice(mc * SN, (mc + 1) * SN)
                        r3p = pp.tile([128, SN], F32, tag="pp")
                        mm(r3p[:], W["ident"][:], h2_sb[:, sl], stop=False)
                        mm(r3p[:], W["res3_w"][:, mc * 128:(mc + 1) * 128],
                           r2_sb[:], start=False)
                        nc.scalar.activation(encf[:, sl], r3p[:], ACTF.Relu,
                                             bias=c0(cn))
                    ctx["encf"] = encf

                def s_pf1():
                    f1p = pp.tile([128, SN], F32, tag="pp")
                    mm(f1p[:], W["pol1_w"][:], x16s)
                    m_f1 = ap.tile([128, SN], F16, tag="m5", bufs=4)
                    sin_red(m_f1[:], f1p[:], c0("pf1"))
                    pf1 = ap.tile([128, SN], F16, tag="a512", bufs=6)
                    nc.scalar.activation(pf1[:], m_f1[:], ACTF.Sin)
                    ctx["pf1"] = pf1

                def s_pf2():
                    f2p = pp.tile([128, SN], F32, tag="pp")
                    mm(f2p[:], W["pol2_w"][:], ctx["pf1"][:])
                    m_f2 = ap.tile([128, SN], F16, tag="m5", bufs=4)
                    sin_red(m_f2[:], f2p[:], c0("pf2"))
                    pf2 = ap.tile([128, SN], F16, tag="a512", bufs=6)
                    nc.scalar.activation(pf2[:], m_f2[:], ACTF.Sin)
                    ctx["pf2"] = pf2

                stages = [s_pe, s_h1, s_h2, s_r12, s_r3, s_pf1, s_pf2]
                if stage_sink is None:
                    # inline (prologue): run the two independent chains
                    # interleaved so the PE is never starved by one chain
                    for s in (s_pe, s_pf1, s_h1, s_pf2, s_h2, s_r12, s_r3):
                        s()
                else:
                    stage_sink.extend(stages)

            def emit_s1(e, ctx):
                """First expert layer: matmuls + range reduction + sin.
                Emitted one expert AHEAD of its s2 consumer so the PE never
                waits on the sine chain (software pipelining)."""
                encf = ctx["encf"]
                m1 = ap.tile([128, 2 * SN], F16, tag="m", bufs=6)
                for mc in range(2):
                    s1p = pp.tile([128, SN], F32, tag="pp")
                    for kc in range(2):
                        off = ((e * 2 + kc) * 2 + mc) * 128
                        mm(s1p[:], W["w1p"][:, off:off + 128],
                           encf[:, kc * SN:(kc + 1) * SN],
                           start=(kc == 0), stop=(kc == 1))
                    sin_red(m1[:, mc * SN:(mc + 1) * SN], s1p[:],
                            c0(f"s1_{e}{'ab'[mc]}"))
                e1 = ap.tile([128, 2 * SN], F16, tag="e", bufs=6)
                nc.scalar.activation(e1[:], m1[:], ACTF.Sin)
                return e1

            def emit_s2_fin(e, e1, pgt):
                m2 = ap.tile([128, 2 * SN], F16, tag="m", bufs=6)
                for mc in range(2):
                    s2p = pp.tile([128, SN], F32, tag="pp")
                    for kc in range(2):
                        off = ((e * 2 + kc) * 2 + mc) * 128
                        mm(s2p[:], W["w2p"][:, off:off + 128],
                           e1[:, kc * SN:(kc + 1) * SN],
                           start=(kc == 0), stop=(kc == 1))
                    sin_red(m2[:, mc * SN:(mc + 1) * SN], s2p[:],
                            c0(f"s2_{e}{'ab'[mc]}"))
                e2 = ap.tile([128, 2 * SN], F16, tag="e", bufs=6)
                nc.scalar.activation(e2[:], m2[:], ACTF.Sin)
                preds = pgt[0:7, :]
                for kc in range(2):
                    blk = e * 2 + kc
                    mm(preds, W["finp"][:, blk * 7:blk * 7 + 7],
                       e2[:, kc * SN:(kc + 1) * SN],
                       start=(e == 0 and kc == 0),
                       stop=(e == 6 and kc == 1))

            def emit_phase2A(q):
                """Softmax numer/denom inputs (ACT+DVE only, no PE)."""
                expq = ap.tile([128, SN], F32, tag="e", bufs=6)
                nc.scalar.activation(expq[:], Lall[q][:], ACTF.Exp,
                                     bias=c0("gb"))
                wq = ap.tile([128, SN], F32, tag="m", bufs=6)
                if flags["fb_any"]:
                    pb = ap.tile([128, SN], F32, tag="h2", bufs=2)
                    nc.vector.tensor_scalar_add(pb[:], Pall[q][:], c0("fb"))
                    nc.vector.tensor_mul(wq[:], pb[:], expq[:])
                else:
                    nc.vector.tensor_mul(wq[:], Pall[q][:], expq[:])
                return expq, wq

            def emit_phase2B(q, expq, wq):
                """Reductions + normalize + store.  Emitted a bit after A so
                the PE never waits on A's exp/mul chain."""
                nump = pp.tile([4, SN], F32, tag="pp")
                denp = pp.tile([4, SN], F32, tag="pp")
                # fp32 matmul moving-operand max is 512: split halves
                for hh in range(2):
                    sl = slice(hh * 512, (hh + 1) * 512)
                    mm(nump[:, sl], W["sumw"][:], wq[:, sl])
                    mm(denp[:, sl], W["sumw"][:], expq[:, sl])
                rec = ap.tile([4, SN], F32, tag="m", bufs=6)
                nc.vector.reciprocal_approx_fast(rec[:], denp[:])
                yq = ap.tile([4, SN], F32, tag="m", bufs=6)
                nc.vector.tensor_mul(yq[:], nump[:], rec[:])
                nc.sync.dma_start(ydram[4 * q:4 * q + 4, :], yq[:])

            p2state = {}
            ctx = {}
            emit_trunk(0, ctx)
            for t in range(NST):
                # gate logits in a rotating pp tile, drained immediately by
                # the Lall copy, so the preds accumulator is the only
                # long-lived PSUM tile and the next super's gate never waits.
                lgt = pp.tile([7, SN], F32, tag="pp")
                lg = lgt[:]
                encf, pf2 = ctx["encf"], ctx["pf2"]
                mm(lg, W["gate_wp"][:, 0:7], encf[:, 0:SN], stop=False)
                mm(lg, W["gate_wp"][:, 7:14], encf[:, SN:2 * SN],
                   start=False, stop=False)
                mm(lg, W["gate_wp"][:, 14:21], pf2[:], start=False)
                q_i, j_i = t // 4, t % 4
                nc.scalar.activation(Lall[q_i][32 * j_i:32 * j_i + 7, :],
                                     lg, ACTF.Copy)
                pgt = pg_pool.tile([7, SN], F32, tag="pg")
                # interleave next super-tile's trunk stages between experts:
                # they are independent work that fills the PE/DVE/ACT gaps
                # while each expert's sin chain drains.
                next_ctx = {}
                stages = []
                if t + 1 < NST:
                    emit_trunk(t + 1, next_ctx, stage_sink=stages)
                e1_q = [emit_s1(0, ctx), emit_s1(1, ctx)]
                for e in range(7):
                    e1_cur = e1_q.pop(0)
                    if e + 2 < 7:
                        e1_q.append(emit_s1(e + 2, ctx))
                    emit_s2_fin(e, e1_cur, pgt)
                    if e == 0 and t == 4 and "pending" in p2state:
                        emit_phase2B(0, *p2state.pop("pending"))
                    if e < len(stages):
                        stages[e]()
                nc.scalar.activation(Pall[q_i][32 * j_i:32 * j_i + 7, :],
                                     pgt[0:7, :], ACTF.Copy)
                ctx = next_ctx
                if t == 3:
                    p2state["pending"] = emit_phase2A(0)
                elif t == NST - 1:
                    expq1, wq1 = emit_phase2A(1)
                    emit_phase2B(1, expq1, wq1)

            def _noop():
                pass

    # populate .instr bytes for InstISA subclasses (custom DVE ops) — Bacc
    # runs this in compile(); the plain Bass+Tile path does not.
    mybir.codegen_inst_isa_subclasses(nc)
    return nc


_BUILD_CACHE = {}


def _get_nc(cols_key, cols, flags):
    key = (cols_key, tuple(sorted(flags.items())))
    if key not in _BUILD_CACHE:
        _BUILD_CACHE[key] = _build(cols, flags)
    return _BUILD_CACHE[key]


def run(inputs, trace=False):
    d, cols, flags = _prepare(inputs)
    nc = _get_nc(len(cols), cols, flags)
    in_maps = []
    for c in range(NCORES):
        m = {k: v for k, v in d.items() if k not in ("x16", "xlo")}
        m["x16"] = np.ascontiguousarray(d["x16"][:, c * BC:(c + 1) * BC])
        m["xlo"] = np.ascontiguousarray(d["xlo"][:, c * BC:(c + 1) * BC])
        in_maps.append(m)
    res = run_bass_kernel_spmd(nc, in_maps, list(range(NCORES)), trace=trace)
    y = np.concatenate([r["y"].reshape(-1) for r in res.results])
    return y.reshape(B, 1).astype(np.float32), res


def kernel(**inputs):
    y, _ = run(inputs, trace=False)
    return y


# revision 25
# speedup vs baseline: 1.0091x; 1.0091x over previous
"""Trainium2 Bass kernel for nn_MoEINR: SIREN MoE implicit neural repr.

Pipeline per point: NeRF positional encoding -> SIREN encoder (2 sine layers +
relu bottleneck residual block) -> policy sine net + softmax gate over 7
experts -> 7 SIREN expert MLPs evaluated densely -> probability-weighted sum.

Strategy: pure data parallel over B=65536 points across 8 cores (8192
points/core, 8 super-tiles of 1024).  All activations feature-major
[feat, batch]; every GEMM is lhsT.T @ rhs with weights pre-transposed and
pre-scaled on the host.  Everything runs fp16 on the PE (1 cycle/row, 1024
moving): x is fed as an fp16 hi+lo pair so even the positional-encoding
matmul (which needs ~22 mantissa bits) is exact via two accumulating fp16
matmuls.  SIREN sines need range reduction (ScalarE Sin table is only valid
in [-pi,pi]): weights are pre-scaled by 30/2pi so matmuls produce q =
z*30/(2pi) in period units, then a fused custom DVE op computes
2pi*(q + b - round(q + b)) via the magic-number rounding trick (fp32 PSUM in,
fp16 out), and ACT Sin evaluates it.  The residual join runs on the PE (fp16
identity matmul accumulated with res3) + ACT relu.  Gate logits and expert
preds share one PSUM tile (partition rows 32:39 / 0:7).  Softmax/exp runs as
a second phase after all tiles so the ACT table set switches only once.
"""
import os
import sys

sys.path.insert(0, "/opt/trn_rl_repo")

import numpy as np

import bass_rust
import concourse.bass as bass
import concourse.mybir as mybir
from concourse import tile
from concourse import dve_ops as dops
from concourse.dve_spec import Spec, Src0, Src1, C0, C1, C2, relu as dve_relu_node
from concourse.dve_uop import DveOpSpec
from concourse.dve_spec import lower as dve_lower, _has_src1 as dve_has_src1
from concourse.bass_utils import run_bass_kernel_spmd

F32 = mybir.dt.float32
F16 = mybir.dt.float16    # 1 cycle/row PE matmul, ~1e-3 rel err (13x margin)
ALU = mybir.AluOpType
ACTF = mybir.ActivationFunctionType

NCORES = 8
B = 65536
BC = B // NCORES          # 8192 points per core
SN = 1024                 # super-tile batch (fp16 moving-operand max)
NST = BC // SN            # super-tiles per core (8)
TWO_PI = float(2.0 * np.pi)
SCL = np.float32(30.0 / TWO_PI)   # radians -> periods prescale for sine layers
MAGIC = np.float32(1.5 * 2 ** 23)  # fp32 round-to-nearest-int via add/sub

# ---------------------------------------------------------------------------
# Tile framework workarounds: this walrus build accepts at most ONE sync-wait
# per instruction; Tile attaches one wait per dependent proc.  Split them.
# ---------------------------------------------------------------------------
_wsplit_counter = [0]


def _split_multiwaits(ordered):
    for bb_name, insts in ordered.items():
        i = 0
        while i < len(insts):
            inst = insts[i]
            si = inst.sync_info
            waits = list(si.on_wait) if si is not None and si.on_wait else []
            if len(waits) > 1:
                keep = waits[-1]
                extras = waits[:-1]
                while len(si.on_wait) > 0:
                    si.on_wait.pop()
                si.on_wait.append(keep)
                for w in extras:
                    _wsplit_counter[0] += 1
                    nop = mybir.InstNoOp(name=f"wsplit-{_wsplit_counter[0]}")
                    nop.engine = inst.engine
                    nop.bass_nofuse = True
                    nop.sync_info = mybir.SyncInfo(on_wait=[w], on_update=[])
                    insts.insert(i, nop)
                    i += 1
            i += 1


class _SplittingClockWait:
    def __init__(self, tc, ordered):
        self._inner = bass_rust.TileClockWait(tc, ordered)
        self._ordered = ordered

    def assign_waits(self, start_bb_name):
        r = self._inner.assign_waits(start_bb_name)
        _split_multiwaits(self._ordered)
        return r

    def __getattr__(self, name):
        return getattr(self._inner, name)


tile.TileClockWait = _SplittingClockWait


class TC(tile.TileContext):
    """TileContext whose tail drain emits one wait per instruction."""

    def _drain_and_barrier(self, tick_clock, wait_clock):
        nc = self.nc
        collector = nc.sync.nop(nofuse=True)
        wait_clock.add_sem_waits(
            collector.ins, bass_rust.ScopedClock({None: tick_clock.global_clock})
        )
        si = collector.ins.sync_info
        waits = list(si.on_wait) if si is not None and si.on_wait else []
        if len(waits) > 1:
            id_to_handle = {h.num: h for h in self.sems.allocated().values()}
            extras = waits[1:]
            while len(si.on_wait) > 1:
                si.on_wait.pop()
            for w in extras:
                assert w.wait_mode == "sem-ge-imm", w.wait_mode
                nc.sync.wait_ge(id_to_handle[w.id], w.wait_value)
        nc.sync.drain()
        nc.all_engine_barrier()
        assert self.sems is not None
        popped = nc._tile_sem_poison_stack.pop()
        assert popped is self._sem_poison
        nc.clear_and_free_semaphores(list(self.sems.allocated().values()))
        nc.all_engine_barrier()


# ---------------------------------------------------------------------------
# Custom DVE op (uop tables are generated at compile time from the Spec).
# SIN_RED: out = (q - ((q + C0) - C0)) * C2  with C0 = MAGIC + bias_periods,
#          C2 = 2pi  ->  full sine-argument range reduction in ONE DVE op.
# ---------------------------------------------------------------------------


def _register_op(name, spec):
    if name in dops._SUB_OPCODE_FOR_NAME:
        return next(o for o in dops.OPS if o.name == name)
    opcode = max(dops._SUB_OPCODE_FOR_NAME.values()) + 1
    assert opcode < 0x20
    op = dops.DveOp(name, spec, subdim=False, uops_sha={})
    for ver in ("v3", "v4"):
        try:
            uops = dve_lower(spec, ver=ver)
        except Exception:
            continue
        s = DveOpSpec(name=name, opcode=opcode, uops=uops,
                      rd1_en=dve_has_src1(spec)).sha(ver)
        op.uops_sha[ver] = s
    dops.OPS.append(op)
    dops.CUSTOM_DVE_SPECS[name] = spec
    dops._SUB_OPCODE_FOR_NAME[name] = opcode
    return op


def _ref_sin_red(in0, in1, s0, s1, imm2):
    p = np.float32(in0.astype(np.float32) + np.float32(s0))
    r = np.float32(np.float32(p + np.float32(s1)) - np.float32(s1))
    return ((p - r) * np.float32(imm2)).astype(np.float32)


_p_node = Src0 + C0   # p = q + bias (C0 = bias AP, C1 = MAGIC imm, C2 = 2pi)
SIN_RED = _register_op(
    "ANT_SIN_RANGE_RED",
    Spec(body=(_p_node - ((_p_node + C1) - C1)) * C2, reference=_ref_sin_red),
)


# ---------------------------------------------------------------------------
# Host-side weight preprocessing
# ---------------------------------------------------------------------------


def _prepare(inputs):
    f = lambda a: np.asarray(a, dtype=np.float32)
    h = np.float16
    d = {}
    xt = np.ascontiguousarray(f(inputs["x"]).T)       # [4,B] fp32
    x16 = xt.astype(h)
    d["x16"] = x16                                    # hi fp16
    d["xlo"] = (xt - x16.astype(np.float32)).astype(h)  # lo fp16 (|.|<2^-12)

    # positional encoding: q[i*16+j] = x_i * 2^(j%8) / 2 (periods);
    # cos rows (j>=8) get +0.25 period via the C0 bias.  Powers of two are
    # exact in fp16; x arrives as hi+lo fp16 so q is fp32-exact.
    pe_w = np.zeros((4, 64), h)
    for i in range(4):
        for j in range(8):
            pe_w[i, i * 16 + j] = 2.0 ** j / 2.0
            pe_w[i, i * 16 + 8 + j] = 2.0 ** j / 2.0
    d["pe_w"] = pe_w

    d["enc1_w"] = np.ascontiguousarray((f(inputs["enc_s1_w"]) * SCL).T.astype(h))
    d["enc2_w"] = np.ascontiguousarray((f(inputs["enc_s2_w"]) * SCL).T.astype(h))
    r1t = f(inputs["res_fc1_w"]).T                                        # [256,128]
    d["res1_w"] = np.ascontiguousarray(
        np.concatenate([r1t[0:128], r1t[128:256]], axis=1).astype(h))     # [128,256]
    d["res2_w"] = np.ascontiguousarray(f(inputs["res_fc2_w"]).T.astype(h))
    d["res3_w"] = np.ascontiguousarray(f(inputs["res_fc3_w"]).T.astype(h))
    d["pol1_w"] = np.ascontiguousarray((f(inputs["pol_s1_w"]) * SCL).T.astype(h))
    d["pol2_w"] = np.ascontiguousarray((f(inputs["pol_s2_w"]) * SCL).T.astype(h))
    gt = f(inputs["gate_w"]).T                                            # [384,7]
    d["gate_wp"] = np.ascontiguousarray(
        np.concatenate([gt[0:128], gt[128:256], gt[256:384]], axis=1).astype(h))
    d["ident"] = np.eye(128, dtype=h)                 # residual add via PE

    w1 = np.zeros((128, 7 * 4 * 128), np.float32)
    w2 = np.zeros((128, 7 * 4 * 128), np.float32)
    for e in range(7):
        t1 = (f(inputs["exp_s1_w"][e]) * SCL).T      # [256,256] (in,out)
        t2 = (f(inputs["exp_s2_w"][e]) * SCL).T
        for kc in range(2):
            for mc in range(2):
                off = ((e * 2 + kc) * 2 + mc) * 128
                w1[:, off:off + 128] = t1[kc * 128:(kc + 1) * 128,
                                          mc * 128:(mc + 1) * 128]
                w2[:, off:off + 128] = t2[kc * 128:(kc + 1) * 128,
                                          mc * 128:(mc + 1) * 128]
    d["w1p"] = w1.astype(h)
    d["w2p"] = w2.astype(h)

    finw = f(inputs["exp_fin_w"])                    # [7,1,256]
    finp = np.zeros((128, 14 * 7), np.float32)
    for e in range(7):
        for kc in range(2):
            blk = e * 2 + kc
            finp[:, blk * 7 + e] = finw[e, 0, kc * 128:(kc + 1) * 128]
    d["finp"] = finp.astype(h)

    sumw = np.zeros((128, 4), np.float32)
    for p in range(128):
        if p % 32 < 7:
            sumw[p, p // 32] = 1.0
    d["sumw"] = sumw

    # C0 constants / biases, packed column-wise into one [128, NCOL] tensor
    bias = {
        "h1": f(inputs["enc_s1_b"]) * SCL,
        "h2": f(inputs["enc_s2_b"]) * SCL,
        "pf1": f(inputs["pol_s1_b"]) * SCL,
        "pf2": f(inputs["pol_s2_b"]) * SCL,
        "r1": f(inputs["res_fc1_b"]),
        "r2": f(inputs["res_fc2_b"]),
        "r3": f(inputs["res_fc3_b"]),
    }
    cols = {}
    cv = []

    def addcol(name, vec128):
        cols[name] = len(cv)
        v = np.zeros(128, np.float32)
        v[: len(vec128)] = vec128
        cv.append(v)

    pe_c0 = np.zeros(64, np.float32)
    for i in range(4):
        pe_c0[i * 16 + 8: i * 16 + 16] += 0.25
    addcol("pe", pe_c0)
    addcol("h1", bias["h1"])
    addcol("h2a", bias["h2"][0:128])
    addcol("h2b", bias["h2"][128:256])
    addcol("pf1", bias["pf1"])
    addcol("pf2", bias["pf2"])
    addcol("r1", bias["r1"])
    addcol("r2", bias["r2"])
    addcol("r3a", bias["r3"][0:128])
    addcol("r3b", bias["r3"][128:256])
    gbr = np.zeros(128, np.float32)
    fbr = np.zeros(128, np.float32)
    gb = f(inputs["gate_b"])
    fb = f(inputs["exp_fin_b"]).reshape(-1)
    for j in range(4):
        gbr[32 * j: 32 * j + 7] = gb
        fbr[32 * j: 32 * j + 7] = fb
    addcol("gb", gbr)
    addcol("fb", fbr)
    e1b = f(inputs["exp_s1_b"]) * SCL                # [7,256]
    e2b = f(inputs["exp_s2_b"]) * SCL
    for e in range(7):
        addcol(f"s1_{e}a", e1b[e, 0:128])
        addcol(f"s1_{e}b", e1b[e, 128:256])
        addcol(f"s2_{e}a", e2b[e, 0:128])
        addcol(f"s2_{e}b", e2b[e, 128:256])
    d["cvec"] = np.ascontiguousarray(np.stack(cv, axis=1))   # [128, ncol]

    flags = {
        "fb_any": bool(np.any(fb != 0)),
    }
    return d, cols, flags


# ---------------------------------------------------------------------------
# Bass kernel builder
# ---------------------------------------------------------------------------


def _build(cols, flags):
    nc = bass.Bass()
    P = {}
    shapes = {
        "x16": [4, BC], "xlo": [4, BC], "pe_w": [4, 64], "enc1_w": [64, 128],
        "enc2_w": [128, 256], "res1_w": [128, 256], "res2_w": [128, 128],
        "res3_w": [128, 256], "pol1_w": [4, 128], "pol2_w": [128, 128],
        "gate_wp": [128, 21], "w1p": [128, 3584], "w2p": [128, 3584],
        "finp": [128, 98], "sumw": [128, 4], "cvec": [128, len(cols)],
        "ident": [128, 128],
    }
    F32_NAMES = ("sumw", "cvec")
    dt_of = {n: (F32 if n in F32_NAMES else F16) for n in shapes}
    for n, s in shapes.items():
        P[n] = nc.dram_tensor(n, s, dt_of[n], kind="ExternalInput")
    ydram = nc.dram_tensor("y", [NST, SN], F32, kind="ExternalOutput")

    with TC(nc) as tc:
        with (
            tc.tile_pool(name="wp", bufs=1) as wp,
            tc.tile_pool(name="ap", bufs=1) as ap,
            tc.tile_pool(name="pp", bufs=3, space="PSUM") as pp,
            tc.tile_pool(name="pg", bufs=1, space="PSUM") as pg_pool,
        ):
            W = {}
            for n in shapes:
                W[n] = wp.tile(shapes[n], dt_of[n], tag=n, name=n)
                nc.sync.dma_start(W[n][:], P[n][:])

            def c0(name, rows=128):
                c = cols[name]
                return W["cvec"][0:rows, c:c + 1]

            Lall = [wp.tile([128, SN], F32, tag=f"Lall{q}", name=f"Lall{q}")
                    for q in range(2)]
            Pall = [wp.tile([128, SN], F32, tag=f"Pall{q}", name=f"Pall{q}")
                    for q in range(2)]
            for q in range(2):
                nc.vector.memset(Lall[q][:], 0.0)
                nc.vector.memset(Pall[q][:], 0.0)

            def mm(out, lhsT, rhs, start=True, stop=True):
                # ISA limit: matmul free dim <= 512 (one PSUM bank of fp32).
                # Split the moving dim; each half is its own accum region so
                # start/stop flags carry over per half.
                n = rhs.shape[-1]
                if n <= 512:
                    nc.tensor.matmul(out, lhsT, rhs, start=start, stop=stop)
                    return
                assert n % 512 == 0
                for hh in range(n // 512):
                    sl = slice(hh * 512, (hh + 1) * 512)
                    nc.tensor.matmul(out[:, sl], lhsT, rhs[:, sl],
                                     start=start, stop=stop)

            def sin_red(m_out, q_psum, c0_ap):
                nc.vector._custom_dve(SIN_RED, out=m_out, in0=q_psum,
                                      s0=c0_ap, s1=float(MAGIC), imm2=TWO_PI)

            def emit_trunk(t, ctx, stage_sink=None):
                """Emit trunk for super-tile t.  If stage_sink is a list,
                append 7 stage closures instead of emitting inline."""
                x16s = W["x16"][:, t * SN:(t + 1) * SN]
                xlos = W["xlo"][:, t * SN:(t + 1) * SN]

                def s_pe():
                    ang = pp.tile([64, SN], F32, tag="pp")
                    mm(ang[:], W["pe_w"][:], x16s, stop=False)
                    mm(ang[:], W["pe_w"][:], xlos, start=False)
                    m_pe = ap.tile([64, SN], F16, tag="m5", bufs=6)
                    sin_red(m_pe[:], ang[:], c0("pe", 64))
                    pe_sb = ap.tile([64, SN], F16, tag="a512", bufs=8)
                    nc.scalar.activation(pe_sb[:], m_pe[:], ACTF.Sin)
                    ctx["pe_sb"] = pe_sb

                def s_h1():
                    h1p = pp.tile([128, SN], F32, tag="pp")
                    mm(h1p[:], W["enc1_w"][:], ctx["pe_sb"][:])
                    m_h1 = ap.tile([128, SN], F16, tag="m5", bufs=6)
                    sin_red(m_h1[:], h1p[:], c0("h1"))
                    h1_sb = ap.tile([128, SN], F16, tag="a512", bufs=8)
                    nc.scalar.activation(h1_sb[:], m_h1[:], ACTF.Sin)
                    ctx["h1_sb"] = h1_sb

                def s_h2():
                    m_h2 = ap.tile([128, 2 * SN], F16, tag="m", bufs=6)
                    for mc, cn in ((0, "h2a"), (1, "h2b")):
                        h2p = pp.tile([128, SN], F32, tag="pp")
                        mm(h2p[:], W["enc2_w"][:, mc * 128:(mc + 1) * 128],
                           ctx["h1_sb"][:])
                        sin_red(m_h2[:, mc * SN:(mc + 1) * SN], h2p[:],
                                c0(cn))
                    h2_sb = ap.tile([128, 2 * SN], F16, tag="h2", bufs=3)
                    nc.scalar.activation(h2_sb[:], m_h2[:], ACTF.Sin)
                    ctx["h2_sb"] = h2_sb

                def s_r12():
                    h2_sb = ctx["h2_sb"]
                    r1p = pp.tile([128, SN], F32, tag="pp")
                    mm(r1p[:], W["res1_w"][:, 0:128], h2_sb[:, 0:SN],
                       stop=False)
                    mm(r1p[:], W["res1_w"][:, 128:256], h2_sb[:, SN:2 * SN],
                       start=False)
                    r1_sb = ap.tile([128, SN], F16, tag="a512", bufs=8)
                    nc.scalar.activation(r1_sb[:], r1p[:], ACTF.Relu,
                                         bias=c0("r1"))
                    r2p = pp.tile([128, SN], F32, tag="pp")
                    mm(r2p[:], W["res2_w"][:], r1_sb[:])
                    r2_sb = ap.tile([128, SN], F16, tag="a512", bufs=8)
                    nc.scalar.activation(r2_sb[:], r2p[:], ACTF.Relu,
                                         bias=c0("r2"))
                    ctx["r2_sb"] = r2_sb

                def s_r3():
                    h2_sb, r2_sb = ctx["h2_sb"], ctx["r2_sb"]
                    encf = ap.tile([128, 2 * SN], F16, tag="encf", bufs=3)
                    for mc, cn in ((0, "r3a"), (1, "r3b")):
                        sl = slice(mc * SN, (mc + 1) * SN)
                        r3p = pp.tile([128, SN], F32, tag="pp")
                        mm(r3p[:], W["ident"][:], h2_sb[:, sl], stop=False)
                        mm(r3p[:], W["res3_w"][:, mc * 128:(mc + 1) * 128],
                           r2_sb[:], start=False)
                        nc.scalar.activation(encf[:, sl], r3p[:], ACTF.Relu,
                                             bias=c0(cn))
                    ctx["encf"] = encf

                def s_pf1():
                    f1p = pp.tile([128, SN], F32, tag="pp")
                    mm(f1p[:], W["pol1_w"][:], x16s)
                    m_f1 = ap.tile([128, SN], F16, tag="m5", bufs=6)
                    sin_red(m_f1[:], f1p[:], c0("pf1"))
                    pf1 = ap.tile([128, SN], F16, tag="a512", bufs=8)
                    nc.scalar.activation(pf1[:], m_f1[:], ACTF.Sin)
                    ctx["pf1"] = pf1

                def s_pf2():
                    f2p = pp.tile([128, SN], F32, tag="pp")
                    mm(f2p[:], W["pol2_w"][:], ctx["pf1"][:])
                    m_f2 = ap.tile([128, SN], F16, tag="m5", bufs=6)
                    sin_red(m_f2[:], f2p[:], c0("pf2"))
                    pf2 = ap.tile([128, SN], F16, tag="a512", bufs=8)
                    nc.scalar.activation(pf2[:], m_f2[:], ACTF.Sin)
                    ctx["pf2"] = pf2

                stages = [s_pe, s_h1, s_h2, s_r12, s_r3, s_pf1, s_pf2]
                if stage_sink is None:
                    for s in stages:
                        s()
                else:
                    stage_sink.extend(stages)

            def emit_s1(e, ctx):
                """First expert layer: matmuls + range reduction + sin.
                Emitted one expert AHEAD of its s2 consumer so the PE never
                waits on the sine chain (software pipelining)."""
                encf = ctx["encf"]
                m1 = ap.tile([128, 2 * SN], F16, tag="m", bufs=6)
                for mc in range(2):
                    s1p = pp.tile([128, SN], F32, tag="pp")
                    for kc in range(2):
                        off = ((e * 2 + kc) * 2 + mc) * 128
                        mm(s1p[:], W["w1p"][:, off:off + 128],
                           encf[:, kc * SN:(kc + 1) * SN],
                           start=(kc == 0), stop=(kc == 1))
                    sin_red(m1[:, mc * SN:(mc + 1) * SN], s1p[:],
                            c0(f"s1_{e}{'ab'[mc]}"))
                e1 = ap.tile([128, 2 * SN], F16, tag="e", bufs=6)
                nc.scalar.activation(e1[:], m1[:], ACTF.Sin)
                return e1

            def emit_s2_fin(e, e1, pgt):
                m2 = ap.tile([128, 2 * SN], F16, tag="m", bufs=6)
                for mc in range(2):
                    s2p = pp.tile([128, SN], F32, tag="pp")
                    for kc in range(2):
                        off = ((e * 2 + kc) * 2 + mc) * 128
                        mm(s2p[:], W["w2p"][:, off:off + 128],
                           e1[:, kc * SN:(kc + 1) * SN],
                           start=(kc == 0), stop=(kc == 1))
                    sin_red(m2[:, mc * SN:(mc + 1) * SN], s2p[:],
                            c0(f"s2_{e}{'ab'[mc]}"))
                e2 = ap.tile([128, 2 * SN], F16, tag="e", bufs=6)
                nc.scalar.activation(e2[:], m2[:], ACTF.Sin)
                preds = pgt[0:7, :]
                for kc in range(2):
                    blk = e * 2 + kc
                    mm(preds, W["finp"][:, blk * 7:blk * 7 + 7],
                       e2[:, kc * SN:(kc + 1) * SN],
                       start=(e == 0 and kc == 0),
                       stop=(e == 6 and kc == 1))

            def emit_phase2A(q):
                """Softmax numer/denom inputs (ACT+DVE only, no PE)."""
                expq = ap.tile([128, SN], F32, tag="e", bufs=6)
                nc.scalar.activation(expq[:], Lall[q][:], ACTF.Exp,
                                     bias=c0("gb"))
                wq = ap.tile([128, SN], F32, tag="m", bufs=6)
                if flags["fb_any"]:
                    pb = ap.tile([128, SN], F32, tag="h2", bufs=3)
                    nc.vector.tensor_scalar_add(pb[:], Pall[q][:], c0("fb"))
                    nc.vector.tensor_mul(wq[:], pb[:], expq[:])
                else:
                    nc.vector.tensor_mul(wq[:], Pall[q][:], expq[:])
                return expq, wq

            def emit_phase2B(q, expq, wq):
                """Reductions + normalize + store.  Emitted a bit after A so
                the PE never waits on A's exp/mul chain."""
                nump = pp.tile([4, SN], F32, tag="pp")
                denp = pp.tile([4, SN], F32, tag="pp")
                # fp32 matmul moving-operand max is 512: split halves
                for hh in range(2):
                    sl = slice(hh * 512, (hh + 1) * 512)
                    mm(nump[:, sl], W["sumw"][:], wq[:, sl])
                    mm(denp[:, sl], W["sumw"][:], expq[:, sl])
                rec = ap.tile([4, SN], F32, tag="m", bufs=6)
                nc.vector.reciprocal_approx_fast(rec[:], denp[:])
                yq = ap.tile([4, SN], F32, tag="m", bufs=6)
                nc.vector.tensor_mul(yq[:], nump[:], rec[:])
                nc.sync.dma_start(ydram[4 * q:4 * q + 4, :], yq[:])

            p2state = {}
            ctx = {}
            emit_trunk(0, ctx)
            for t in range(NST):
                # gate logits + expert preds share one PSUM tile
                pgt = pg_pool.tile([64, SN], F32, tag="pg")
                lg = pgt[32:39, :]
                encf, pf2 = ctx["encf"], ctx["pf2"]
                mm(lg, W["gate_wp"][:, 0:7], encf[:, 0:SN], stop=False)
                mm(lg, W["gate_wp"][:, 7:14], encf[:, SN:2 * SN],
                   start=False, stop=False)
                mm(lg, W["gate_wp"][:, 14:21], pf2[:], start=False)
                # interleave next super-tile's trunk stages between experts:
                # they are independent work that fills the PE/DVE/ACT gaps
                # while each expert's sin chain drains.
                next_ctx = {}
                stages = []
                if t + 1 < NST:
                    emit_trunk(t + 1, next_ctx, stage_sink=stages)
                e1_q = [emit_s1(0, ctx), emit_s1(1, ctx)]
                for e in range(7):
                    e1_cur = e1_q.pop(0)
                    if e + 2 < 7:
                        e1_q.append(emit_s1(e + 2, ctx))
                    emit_s2_fin(e, e1_cur, pgt)
                    if e == 0 and t == 4 and "pending" in p2state:
                        emit_phase2B(0, *p2state.pop("pending"))
                    if e < len(stages):
                        stages[e]()
                q_i, j_i = t // 4, t % 4
                # staging copies on ACT (Copy shares the trig table set; DVE
                # is the busiest engine, ACT has slack)
                nc.scalar.activation(Lall[q_i][32 * j_i:32 * j_i + 7, :],
                                     lg, ACTF.Copy)
                nc.scalar.activation(Pall[q_i][32 * j_i:32 * j_i + 7, :],
                                     pgt[0:7, :], ACTF.Copy)
                ctx = next_ctx
                if t == 3:
                    p2state["pending"] = emit_phase2A(0)
                elif t == NST - 1:
                    expq1, wq1 = emit_phase2A(1)
                    emit_phase2B(1, expq1, wq1)

            def _noop():
                pass

    # populate .instr bytes for InstISA subclasses (custom DVE ops) — Bacc
    # runs this in compile(); the plain Bass+Tile path does not.
    mybir.codegen_inst_isa_subclasses(nc)
    return nc


_BUILD_CACHE = {}


def _get_nc(cols_key, cols, flags):
    key = (cols_key, tuple(sorted(flags.items())))
    if key not in _BUILD_CACHE:
        _BUILD_CACHE[key] = _build(cols, flags)
    return _BUILD_CACHE[key]


def run(inputs, trace=False):
    d, cols, flags = _prepare(inputs)
    nc = _get_nc(len(cols), cols, flags)
    in_maps = []
    for c in range(NCORES):
        m = {k: v for k, v in d.items() if k not in ("x16", "xlo")}
        m["x16"] = np.ascontiguousarray(d["x16"][:, c * BC:(c + 1) * BC])
        m["xlo"] = np.ascontiguousarray(d["xlo"][:, c * BC:(c + 1) * BC])
        in_maps.append(m)
    res = run_bass_kernel_spmd(nc, in_maps, list(range(NCORES)), trace=trace)
    y = np.concatenate([r["y"].reshape(-1) for r in res.results])
    return y.reshape(B, 1).astype(np.float32), res


def kernel(**inputs):
    y, _ = run(inputs, trace=False)
    return y
